# revision 3
# baseline (speedup 1.0000x reference)
"""Trainium2 Bass kernel for the relational GCN layer (gnn_message_passing).

Math (from the reference):
    out[n, e, i] = sum_k sum_m sum_d adj[n, m, k] * x[m, d, (i-k)%4] * W[d, e, k]

Factored for the PE (contraction dim must sit on SBUF partitions):
    X4[m, f]   = x.reshape(4096, 128)            with f = d*4 + j
    G_k[f, n]  = sum_m X4[m, f] * adj[n, m, k]   (the big 256 MB contraction)
    outT[c, n] = sum_k sum_f Wbig[f, k, c] * G_k[f, n]   with c = e*4 + i
    Wbig[d*4+j, k, e*4+i] = W[d, e, k] if j == (i-k)%4 else 0

Sharding: 1D over the node (row) dim of adj/out — core c owns rows
[c*512, (c+1)*512).  x and the (tiny) weight are replicated.  adj is
pre-packed on the host into the layout the device streams directly as
matmul moving operands ([m-partition, n-free] tiles, contiguous 32 KB per
partition per DMA) so the kernel runs at the HBM roofline with zero
on-chip transposes.
"""

import numpy as np

N_CORES = 8
NODES = 4096
N_PER_CORE = NODES // N_CORES          # 512
F = 128                                # d*4+j
C = 128                                # e*4+i
MB = 32                                # m-chunks of 128 (4096 / 128)
GROUPS = 8                             # DMA groups of 4 m-chunks (4 MB each)
MB_PER_GROUP = MB // GROUPS            # 4
R = 4

_PATCHED = False
_PROG = None


def _patch_tile_drain():
    """This container's walrus build rejects >2 sync waits on one Drain;
    split the Tile end-of-context drain into one single-wait drain per proc
    (semantically identical: the SP engine observes each clock lane in
    sequence before the barrier)."""
    global _PATCHED
    if _PATCHED:
        return
    from concourse.tile import TileContext
    from concourse.vector_clock import ScopedClock, VectorClock
    from concourse.tile_scheduler import N_PROCS

    def _split_drain_and_barrier(self, tick_clock, wait_clock):
        g = tick_clock.global_clock
        for p in range(N_PROCS):
            if g[p] > 0:
                d = self.nc.sync.drain()
                pc = VectorClock([g[q] if q == p else 0 for q in range(N_PROCS)])
                wait_clock.add_sem_waits(d.ins, ScopedClock({None: pc}))
        self.nc.all_engine_barrier()
        assert self.sems is not None
        popped = self.nc._tile_sem_poison_stack.pop()
        assert popped is self._sem_poison
        self.nc.clear_and_free_semaphores(list(self.sems.allocated().values()))
        self.nc.all_engine_barrier()

    TileContext._drain_and_barrier = _split_drain_and_barrier
    _PATCHED = True


def _split_sync_waits(bir_bytes, max_waits=1):
    """This container's walrus build rejects instructions carrying more than
    ~2 sync waits.  Hoist all but one wait of any instruction onto standalone
    EventSemaphore instructions on the same engine immediately before it —
    the engine then observes the semaphores sequentially, which is
    semantically identical."""
    import json
    j = json.loads(bir_bytes)
    n_new = 0
    for f in j.get("functions", []):
        for bb in f.get("blocks", []):
            out_insts = []
            for inst in bb.get("instructions", []):
                waits = (inst.get("sync_info") or {}).get("on_wait") or []
                if len(waits) > max_waits:
                    keep = waits[-max_waits:]
                    for w in waits[:-max_waits]:
                        n_new += 1
                        ev = {
                            "engine": inst["engine"],
                            "ins": [],
                            "name": f"{inst['name']}_wsplit{n_new}",
                            "opcode": "EventSemaphore",
                            "outs": [],
                            "sync_info": {"on_update": [], "on_wait": [w]},
                        }
                        if "debug" in inst:
                            ev["debug"] = inst["debug"]
                        out_insts.append(ev)
                    inst["sync_info"]["on_wait"] = keep
                out_insts.append(inst)
            bb["instructions"] = out_insts
    return json.dumps(j).encode()


def _build_program():
    global _PROG
    if _PROG is not None:
        return _PROG
    _patch_tile_drain()
    import concourse.bass as bass
    import concourse.mybir as mybir
    from concourse.tile import TileContext

    f32 = mybir.dt.float32
    nc = bass.Bass()
    # adjt[g, mp, a, k, nn] = adj[n0+nn, (4g+a)*128+mp, k]; flat free dim 8192
    adjt = nc.dram_tensor("adjt", [GROUPS, 128, MB_PER_GROUP * R * N_PER_CORE], f32,
                          kind="ExternalInput")
    # x4t[mp, mb, f] = x.reshape(4096, 128)[mb*128+mp, f]
    x4t = nc.dram_tensor("x4t", [128, MB, F], f32, kind="ExternalInput")
    # wbigt[f, k, c]
    wbigt = nc.dram_tensor("wbigt", [F, R, C], f32, kind="ExternalInput")
    outt = nc.dram_tensor("outt", [C, N_PER_CORE], f32, kind="ExternalOutput")

    with TileContext(nc) as tc:
        with (
            tc.tile_pool(name="const", bufs=1) as cpool,
            tc.tile_pool(name="adj", bufs=3) as apool,
            tc.tile_pool(name="gout", bufs=1) as gpool,
            tc.tile_pool(name="psum", bufs=1, space="PSUM") as ppool,
        ):
            x4sb = cpool.tile([128, MB, F], f32)
            nc.sync.dma_start(out=x4sb[:, :, :], in_=x4t[:, :, :])
            wsb = cpool.tile([F, R, C], f32)
            nc.sync.dma_start(out=wsb[:, :, :], in_=wbigt[:, :, :])

            gps = [ppool.tile([F, N_PER_CORE], f32, tag=f"g{k}", name=f"gps{k}")
                   for k in range(R)]

            for g in range(GROUPS):
                adjsb = apool.tile([128, MB_PER_GROUP * R * N_PER_CORE], f32)
                nc.sync.dma_start(out=adjsb[:, :], in_=adjt[g, :, :])
                for a in range(MB_PER_GROUP):
                    mb = g * MB_PER_GROUP + a
                    for k in range(R):
                        off = (a * R + k) * N_PER_CORE
                        nc.tensor.matmul(
                            gps[k][:, :],
                            lhsT=x4sb[:, mb, :],
                            rhs=adjsb[:, off:off + N_PER_CORE],
                            start=(mb == 0),
                            stop=(mb == MB - 1),
                        )

            gsb = gpool.tile([F, R, N_PER_CORE], f32)
            for k in range(R):
                nc.vector.tensor_copy(gsb[:, k, :], gps[k][:, :])

            ops = ppool.tile([C, N_PER_CORE], f32, tag="out")
            for k in range(R):
                nc.tensor.matmul(
                    ops[:, :],
                    lhsT=wsb[:, k, :],
                    rhs=gsb[:, k, :],
                    start=(k == 0),
                    stop=(k == R - 1),
                )
            osb = gpool.tile([C, N_PER_CORE], f32, tag="osb")
            nc.vector.tensor_copy(osb[:, :], ops[:, :])
            nc.sync.dma_start(out=outt[:, :], in_=osb[:, :])

    _orig_to_json = nc.to_json_bytes
    nc.to_json_bytes = lambda: _split_sync_waits(_orig_to_json())

    _PROG = nc
    return nc


def _pack_adj(adj):
    """adj [4096, 4096, 4] -> per-core [GROUPS, 128, 8192] with
    adjt[c][g, mp, (a, k, nn)] = adj[c*512+nn, (4g+a)*128+mp, k]."""
    A = adj.reshape(N_CORES, N_PER_CORE, GROUPS, MB_PER_GROUP, 128, R)
    At = np.ascontiguousarray(A.transpose(0, 2, 4, 3, 5, 1))
    return At.reshape(N_CORES, GROUPS, 128, MB_PER_GROUP * R * N_PER_CORE)


def kernel(x, adj, weight):
    x = np.ascontiguousarray(np.asarray(x), dtype=np.float32)
    adj = np.ascontiguousarray(np.asarray(adj), dtype=np.float32)
    weight = np.ascontiguousarray(np.asarray(weight), dtype=np.float32)

    x4t = np.ascontiguousarray(
        x.reshape(MB, 128, F).transpose(1, 0, 2))          # [mp, mb, f]
    wbigt = np.zeros((F, R, C), np.float32)                # [f, k, c]
    for k in range(R):
        for i in range(R):
            j = (i - k) % R
            wbigt[j::R, k, i::R] = weight[:, :, k]
    adjt = _pack_adj(adj)

    nc = _build_program()
    from concourse.bass_utils import run_bass_kernel_spmd
    in_maps = [{"adjt": adjt[c], "x4t": x4t, "wbigt": wbigt}
               for c in range(N_CORES)]
    res = run_bass_kernel_spmd(nc, in_maps, core_ids=list(range(N_CORES)))

    outt = np.stack([r["outt"] for r in res.results])      # [8, 128, 512]
    out = outt.reshape(N_CORES, 32, R, N_PER_CORE)         # [c, e, i, nn]
    out = out.transpose(0, 3, 1, 2).reshape(NODES, 32, R)  # [n, e, i]
    return np.ascontiguousarray(out)
